# revision 40
# baseline (speedup 1.0000x reference)
"""2-layer GAT (gnn_message_passing) on 8 Trainium2 NeuronCores.

Strategy (per sharding hint): nodes are partitioned contiguously across the 8
cores (12500 each). Edges (incl. self-loops) are sharded by destination core,
sorted by destination window (128 dst nodes) and source range (quarter of the
node space, so gather indices fit int16), and padded to a per-(window,range)
variable tile grid (tiles = max over cores of ceil(count/128), so all cores
share one SPMD program).

Per layer: a dense phase computes per-node transformed features hp = x @ W
(and the al_dst attention-logit half, folded into the weight matrix), writes
hp as fp8 into a 256B-strided row table and al_dst into a bf16 side table,
and an AllGather replicates the hp table to every core. The edge phase
gathers fp8 hp rows by edge source and bf16 al_dst by edge destination
(custom Ant dma_gather; the per-queue SDMA drain rate for small random reads
is the kernel bottleneck, so payloads are minimized — al_src is recomputed
on-device from the gathered fp8 rows instead of being gathered — and the
gather stream is byte-balanced across all 4 SWDGE queues). It then forms
ee = exp(leaky_relu(al_s + al_d)) per edge, multiplies messages, and
scatter-adds per destination window with a one-hot selection matmul on the PE
(which also accumulates the softmax denominators). Normalization / ELU /
log_softmax tails are batched per span of windows; output rows are contiguous
per window, so no scatter is needed on the way out.
"""
import math
import numpy as np
import ml_dtypes

import concourse.bacc as bacc
import concourse.mybir as mybir
import concourse.tile as tile
from concourse import ap_utils

bf16 = ml_dtypes.bfloat16
F32 = mybir.dt.float32
BF16 = mybir.dt.bfloat16
FP8 = mybir.dt.float8e4
I16 = mybir.dt.int16
I32 = mybir.dt.int32

P = 128
MAX_IDX_PER_GATHER = 3840   # DMA desc ring: <=~4080 idxs per gather inst
SLOPE = 0.2
NQ = 4                      # SWDGE queues for gathers


# ---------------------------------------------------------------- dma_gather
def dma_gather_raw(eng, out_ap, in_ap, idxs_ap, num_idxs, elem_size,
                   elem_step=None, queue_num=0):
    """BassGpSimd.dma_gather (DRAM src, non-transpose) minus the
    elem_size%256B assert (transpose-only restriction, see q7 source) and
    with single_packet=False (large single packets wedge the SDMA)."""
    assert idxs_ap.dtype == mybir.dt.int16
    assert in_ap.dtype == out_ap.dtype
    elem_size_bytes = elem_size * mybir.dt.size(in_ap.dtype)
    assert elem_size_bytes > 0
    if elem_step is None:
        elem_step = elem_size
    assert ap_utils.ap_is_contiguous(in_ap.ap[1:])
    assert ap_utils.ap_is_contiguous(out_ap.ap[1:])
    assert ap_utils.ap_is_contiguous(idxs_ap.ap[1:])
    assert in_ap.ap[0][0] == elem_step
    assert in_ap.ap[-1][1] == elem_size
    assert out_ap.ap[-1][1] == elem_size
    assert num_idxs <= MAX_IDX_PER_GATHER + 256
    stride_bytes = elem_step * mybir.dt.size(in_ap.dtype)
    assert stride_bytes % 256 == 0 and stride_bytes // 256 < 256
    _in_ap = eng.lower_ap_dma(in_ap, for_custom_bir_dma=True)
    _idxs_ap = eng.lower_ap(idxs_ap)
    _out_ap = eng.lower_ap(out_ap)
    return eng.add_instruction(
        mybir.InstDMAGatherAnt(
            name=eng.bass.get_next_instruction_name(),
            ins=[*_in_ap, _idxs_ap, eng.lower_val_access(eng.to_reg(num_idxs))],
            outs=[_out_ap],
            transpose=False,
            num_idxs=num_idxs,
            elem_size=elem_size,
            stride_bytes_256=stride_bytes // 256,
            gen_mode=0,
            single_packet=False,
            queue_num=queue_num,
            sbuf_tokens_per_rank=0,
            sbuf_free_dim_per_rank=0,
            sbuf_free_dim_pad_per_rank=0,
            sbuf_byte_offset=0,
        )
    )


# ------------------------------------------------------------- host preproc
def _wrap_idx(seq16):
    """[L] int -> [128, L//16] int16 in dma_gather idx layout
    (idx j at lane j%16 col j//16, replicated to 8 lane groups)."""
    L = seq16.shape[0]
    w = seq16.reshape(L // 16, 16).T                           # [16,K]
    w = np.tile(w, (8, 1))                                     # [128,K]
    return np.ascontiguousarray(w.astype(np.int16))


class Grid:
    """Static (shared across cores) variable-tile grid for the edge phase."""

    def __init__(self, counts, nwin, nrange):
        # counts: [ncores, nwin, nrange]
        self.nwin, self.nrange = nwin, nrange
        cmax = counts.max(axis=0)                              # [nwin,nrange]
        self.tiles = np.maximum(1, np.ceil(cmax / P).astype(np.int64))
        self.wtiles = self.tiles.sum(axis=1)                   # [nwin]
        # greedy window spans: per-range idx count within a span <= cap
        cap_tiles = MAX_IDX_PER_GATHER // P
        spans = []
        w0 = 0
        while w0 < nwin:
            w1 = w0 + 1
            while w1 < nwin:
                nxt = self.tiles[w0:w1 + 1].sum(axis=0).max()
                if nxt > cap_tiles:
                    break
                w1 += 1
            spans.append((w0, w1))
            w0 = w1
        self.spans = spans
        # per-range per-window tile offset within the whole layout
        self.r_off = np.zeros((nwin, nrange), np.int64)        # in tiles
        for r in range(nrange):
            self.r_off[1:, r] = np.cumsum(self.tiles[:-1, r])
        self.r_tot = self.tiles.sum(axis=0)                    # [nrange]
        # dstrel per-window offsets (in tiles, over concat of all ranges)
        self.w_off = np.zeros(nwin, np.int64)
        self.w_off[1:] = np.cumsum(self.wtiles[:-1])
        self.w_tot = int(self.wtiles.sum())
        self.max_wtiles = int(self.wtiles.max())


def preprocess(edge_index, cfg):
    """Sort/pad edges into the variable (core, window, range, tile) grid."""
    N, ncores, nloc, nwin, nrange = (cfg["N"], cfg["ncores"], cfg["nloc"],
                                     cfg["nwin"], cfg["nrange"])
    rng_sz = N // nrange
    loops = np.arange(N, dtype=np.int64)
    src = np.concatenate([edge_index[0].astype(np.int64), loops])
    dst = np.concatenate([edge_index[1].astype(np.int64), loops])
    core = dst // nloc
    dst_loc = dst - core * nloc
    w = dst_loc // P
    dst_rel = dst_loc - w * P
    r = src // rng_sz
    src_rel = src - r * rng_sz
    key = (core * nwin + w) * nrange + r
    counts = np.bincount(key, minlength=ncores * nwin * nrange).reshape(
        ncores, nwin, nrange)
    grid = Grid(counts, nwin, nrange)

    order = np.argsort(key, kind="stable")
    ks = key[order]
    starts = np.zeros(ncores * nwin * nrange + 1, np.int64)
    np.cumsum(counts.reshape(-1), out=starts[1:])
    pos = np.arange(len(ks)) - starts[ks]
    # window-local tile offset of range r: sum of tiles[w, :r]
    tloc = np.zeros((nwin, nrange), np.int64)
    tloc[:, 1:] = np.cumsum(grid.tiles[:, :-1], axis=1)
    # per (r) gather layout: [r_tot[r]*P] slots; edges of (c,w,r) at
    # r_off[w,r]*P + pos
    per_core = []
    srcrel_o = src_rel[order]
    dstrel_o = dst_rel[order]
    core_o = core[order]
    w_o = w[order]
    r_o = r[order]
    for c in range(ncores):
        m = {}
        sel = core_o == c
        sw, sr, sp = w_o[sel], r_o[sel], pos[sel]
        ssrc, sdst = srcrel_o[sel], dstrel_o[sel]
        for rr in range(nrange):
            rs = sr == rr
            tot = int(grid.r_tot[rr]) * P
            slot = grid.r_off[sw[rs], rr] * P + sp[rs]
            buf = np.zeros(tot, np.int16)
            buf[slot] = ssrc[rs].astype(np.int16)
            m[f"iA{rr}"] = _wrap_idx(buf)
            bufd = np.zeros(tot, np.int16)
            bufd[slot] = (sw[rs] * P + sdst[rs]).astype(np.int16)
            m[f"iB{rr}"] = _wrap_idx(bufd)
        # dstrel layout [128, w_tot] (per window: ranges concat in r order)
        dr = np.full(grid.w_tot * P, -1.0, np.float32)
        gslot = (grid.w_off[sw] + tloc[sw, sr]) * P + sp
        dr[gslot] = sdst.astype(np.float32)
        # device layout [128, w_tot]: slot (tile t, lane p) -> [p, t]
        m["dstrel"] = np.ascontiguousarray(
            dr.reshape(grid.w_tot, P).T.astype(bf16))
        per_core.append(m)
    return per_core, grid


# ------------------------------------------------------------- device build
def build_nc(cfg, grid):
    N, ncores, nloc, nwin, nrange = (cfg["N"], cfg["ncores"], cfg["nloc"],
                                     cfg["nwin"], cfg["nrange"])
    F_IN, H1, C1, C2 = cfg["F_IN"], cfg["H1"], cfg["C1"], cfg["C2"]
    D1 = H1 * C1                   # 64
    A1 = D1 + 2 * H1               # 80: [hp | al_s | al_d]
    T1W = D1 + H1                  # 72 table row (hp | al_s)
    D2 = C2                        # 16
    T2W = D2 + 2                   # 18 table row (hp2 | al_s2 | pad)
    kchunks = F_IN // P
    last_rows = nloc - (nwin - 1) * P
    tiles, spans = grid.tiles, grid.spans
    tloc = np.zeros((nwin, nrange), np.int64)
    tloc[:, 1:] = np.cumsum(tiles[:, :-1], axis=1)

    nc = bacc.Bacc("TRN2", target_bir_lowering=False, num_devices=ncores,
                   num_swdge_queues=NQ)
    xT = nc.dram_tensor("xT", [F_IN, nloc], BF16, kind="ExternalInput")
    W1e = nc.dram_tensor("W1e", [F_IN, A1], BF16, kind="ExternalInput")
    W2e = nc.dram_tensor("W2e", [D1, T2W], BF16, kind="ExternalInput")
    b1r = nc.dram_tensor("b1r", [P, D1], F32, kind="ExternalInput")
    b2r = nc.dram_tensor("b2r", [P, D2], F32, kind="ExternalInput")
    a1sr = nc.dram_tensor("a1sr", [P, D1], F32, kind="ExternalInput")
    a2sr = nc.dram_tensor("a2sr", [P, D2], F32, kind="ExternalInput")
    iA = [nc.dram_tensor(f"iA{rr}", [P, int(grid.r_tot[rr]) * 8], I16,
                         kind="ExternalInput") for rr in range(nrange)]
    iB = [nc.dram_tensor(f"iB{rr}", [P, int(grid.r_tot[rr]) * 8], I16,
                         kind="ExternalInput") for rr in range(nrange)]
    dstrel = nc.dram_tensor("dstrel", [P, grid.w_tot], BF16,
                            kind="ExternalInput")
    out = nc.dram_tensor("out", [nloc, D2], F32, kind="ExternalOutput")

    qbytes = [0] * NQ

    def next_q(nbytes):
        q = min(range(NQ), key=lambda i: qbytes[i])
        qbytes[q] += nbytes
        return q

    with tile.TileContext(nc) as tc:
        with (
            tc.tile_pool(name="const", bufs=1) as cpool,
            tc.tile_pool(name="sbuf", bufs=2) as sb,
            tc.tile_pool(name="gat", bufs=3) as gp,
            tc.tile_pool(name="psum", bufs=2, space="PSUM") as ps,
            tc.tile_pool(name="psum1", bufs=2, space="PSUM") as ps1,
            tc.tile_pool(name="dram", bufs=1, space="DRAM") as dr,
        ):
            # layer-1 table row (256B stride): [64 fp8 hp | 8 bf16 al_s | pad]
            t1loc = dr.tile([nloc, 2 * P], FP8)
            t1full = dr.tile([N, 2 * P], FP8, addr_space="Shared")
            ald1 = dr.tile([nloc, 2 * P], FP8)
            # layer-2 table row (256B stride): [16 fp8 hp2 | 1 bf16 al_s2]
            t2loc = dr.tile([nloc, 2 * P], FP8)
            t2full = dr.tile([N, 2 * P], FP8, addr_space="Shared")
            ald2 = dr.tile([nloc, P], BF16)

            # ---- static constants
            w1s = cpool.tile([P, kchunks, A1], BF16)
            nc.sync.dma_start(out=w1s[:], in_=W1e[:].rearrange(
                "(c p) a -> p c a", p=P))
            w2s = cpool.tile([D1, T2W], BF16)
            nc.sync.dma_start(out=w2s[:], in_=W2e[:])
            b1s = cpool.tile([P, D1], F32)
            nc.sync.dma_start(out=b1s[:], in_=b1r[:])
            b2s = cpool.tile([P, D2], F32)
            nc.sync.dma_start(out=b2s[:], in_=b2r[:])
            a1ss = cpool.tile([P, D1], F32)
            nc.sync.dma_start(out=a1ss[:], in_=a1sr[:])
            a2ss = cpool.tile([P, D2], F32)
            nc.sync.dma_start(out=a2ss[:], in_=a2sr[:])
            iota_i = cpool.tile([P, grid.max_wtiles, P], I32)
            nc.gpsimd.iota(iota_i[:], pattern=[[0, grid.max_wtiles], [1, P]],
                           base=0, channel_multiplier=0)
            iota_f = cpool.tile([P, grid.max_wtiles, P], BF16)
            nc.vector.tensor_copy(out=iota_f[:], in_=iota_i[:])
            ident = cpool.tile([P, P], F32)
            from concourse.masks import make_identity
            make_identity(nc, ident[:])

            # preload all gather idx tables + dstrel (shared by both layers)
            iAs, iBs = [], []
            for rr in range(nrange):
                kr = int(grid.r_tot[rr]) * 8
                ta = cpool.tile([P, kr], I16, name=f"iAs{rr}")
                nc.sync.dma_start(out=ta[:], in_=iA[rr][:])
                iAs.append(ta)
                tb = cpool.tile([P, kr], I16, name=f"iBs{rr}")
                nc.sync.dma_start(out=tb[:], in_=iB[rr][:])
                iBs.append(tb)
            dres = cpool.tile([P, grid.w_tot], BF16)
            nc.sync.dma_start(out=dres[:], in_=dstrel[:])

            # ---- phase A: tables for layer 1
            for b in range(nwin):
                r0 = b * P
                rows = P if b < nwin - 1 else last_rows
                xa = sb.tile([P, kchunks, P], BF16, tag="xa")
                nc.sync.dma_start(
                    out=xa[:, :, :rows],
                    in_=xT[:, r0:r0 + rows].rearrange("(c p) r -> p c r", p=P))
                pA = ps.tile([P, A1], F32, tag="pA")
                for c in range(kchunks):
                    nc.tensor.matmul(pA[:], lhsT=xa[:, c, :], rhs=w1s[:, c, :],
                                     start=(c == 0), stop=(c == kchunks - 1))
                t1row8 = sb.tile([P, D1], FP8, tag="t1row8")
                nc.vector.tensor_copy(out=t1row8[:], in_=pA[:, 0:D1])
                a1row = sb.tile([P, H1], FP8, tag="a1row")
                nc.scalar.copy(out=a1row[:], in_=pA[:, T1W:A1])
                nc.sync.dma_start(out=t1loc[r0:r0 + rows, 0:D1],
                                  in_=t1row8[:rows, :])
                nc.sync.dma_start(out=ald1[r0:r0 + rows, 0:H1],
                                  in_=a1row[:rows, :])

            # ---- allgather T1
            nc.gpsimd.collective_compute(
                "AllGather", mybir.AluOpType.bypass,
                replica_groups=[list(range(ncores))],
                ins=[t1loc[:].opt()], outs=[t1full[:].opt()])

            # ---- edge phases
            def edge_phase(layer):
                tfull = t1full if layer == 1 else t2full
                ald = ald1 if layer == 1 else ald2
                DH = D1 if layer == 1 else D2       # message width
                NH = H1 if layer == 1 else 1        # heads
                CH = DH // NH
                asrc = a1ss if layer == 1 else a2ss
                # fp8 table rows: [DH fp8 hp]; al_s recomputed on-device
                TW = DH
                ebytes = TW
                rng_rows = N // nrange
                for (w0, w1) in spans:
                    span_t = tiles[w0:w1].sum(axis=0)        # per-range tiles
                    rhs_g = []
                    for rr in range(nrange):
                        st = int(span_t[rr])
                        nidx = st * P
                        t_off = int(grid.r_off[w0, rr])      # tiles
                        hg = gp.tile([P, st, TW], FP8, tag=f"hg{rr}")
                        dma_gather_raw(
                            nc.gpsimd, hg[:],
                            tfull[rr * rng_rows:(rr + 1) * rng_rows, 0:TW],
                            iAs[rr][:, t_off * 8:(t_off + st) * 8],
                            nidx, TW, elem_step=2 * P,
                            queue_num=next_q(nidx * ebytes))
                        adt = FP8 if layer == 1 else BF16
                        ag = gp.tile([P, st, NH], adt, tag=f"ag{rr}")
                        dma_gather_raw(
                            nc.gpsimd, ag[:],
                            ald[:, 0:NH],
                            iBs[rr][:, t_off * 8:(t_off + st) * 8],
                            nidx, NH,
                            elem_step=2 * P if layer == 1 else P,
                            queue_num=next_q(nidx * NH * mybir.dt.size(adt)))
                        # al_s = sum_c hp[h,c]*a_src[h,c] from the fp8 rows
                        # (msg tile doubles as scratch for the products)
                        msg = gp.tile([P, st, DH + NH], BF16, tag=f"msg{rr}")
                        nc.vector.tensor_tensor(
                            out=msg[:, :, 0:DH].rearrange(
                                "p t (h c) -> p t h c", h=NH),
                            in0=hg[:, :, 0:DH].rearrange(
                                "p t (h c) -> p t h c", h=NH),
                            in1=asrc[:, None, :].rearrange(
                                "p t (h c) -> p t h c", h=NH).to_broadcast(
                                [P, st, NH, CH]),
                            op=mybir.AluOpType.mult)
                        z = gp.tile([P, st, NH], BF16, tag=f"zz{rr}")
                        with nc.allow_low_precision(
                                reason="8-wide al_s dot; bf16 ee is enough"):
                            nc.vector.tensor_reduce(
                                out=z[:, :, :, None],
                                in_=msg[:, :, 0:DH].rearrange(
                                    "p t (h c) -> p t h c", h=NH),
                                axis=mybir.AxisListType.X,
                                op=mybir.AluOpType.add)
                        # ee = exp(lrelu(al_s + al_d))
                        nc.vector.tensor_tensor(
                            out=z[:], in0=z[:], in1=ag[:],
                            op=mybir.AluOpType.add)
                        zf = z[:].rearrange("p t h -> p (t h)")
                        nc.scalar.activation(
                            out=zf, in_=zf,
                            func=mybir.ActivationFunctionType.Lrelu,
                            alpha=SLOPE)
                        nc.scalar.activation(
                            out=zf, in_=zf,
                            func=mybir.ActivationFunctionType.Exp)
                        # messages into msg: [hp*ee | ee]
                        nc.vector.tensor_tensor(
                            out=msg[:, :, 0:DH].rearrange(
                                "p t (h c) -> p t h c", h=NH),
                            in0=hg[:, :, 0:DH].rearrange(
                                "p t (h c) -> p t h c", h=NH),
                            in1=z[:, :, :, None].to_broadcast(
                                [P, st, NH, CH]),
                            op=mybir.AluOpType.mult)
                        nc.vector.tensor_copy(
                            out=msg[:, :, DH:DH + NH], in_=z[:])
                        rhs_g.append(msg)
                    d_off = int(grid.w_off[w0])
                    dre = dres

                    nw = w1 - w0
                    accb = sb.tile([P, nw, DH + NH], F32, tag="accb")
                    for w in range(w0, w1):
                        wl = w - w0
                        wt = int(grid.wtiles[w])
                        sel = sb.tile([P, wt, P], BF16, tag="sel")
                        dre_o = int(grid.w_off[w])
                        nc.vector.tensor_tensor(
                            out=sel[:],
                            in0=iota_f[:, :wt, :],
                            in1=dre[:, dre_o:dre_o + wt, None].to_broadcast(
                                [P, wt, P]),
                            op=mybir.AluOpType.is_equal)
                        acc = ps.tile([P, DH + NH], F32, tag="acc")
                        nmm = wt
                        i = 0
                        for rr in range(nrange):
                            st_w = int(tiles[w, rr])
                            gt0 = int(grid.r_off[w, rr] - grid.r_off[w0, rr])
                            sl0 = int(tloc[w, rr])
                            for t in range(st_w):
                                nc.tensor.matmul(
                                    acc[:],
                                    lhsT=sel[:, sl0 + t, :],
                                    rhs=rhs_g[rr][:, gt0 + t, 0:DH + NH],
                                    start=(i == 0), stop=(i == nmm - 1))
                                i += 1
                        nc.scalar.copy(out=accb[:, wl, :], in_=acc[:])

                    # span-batched normalization + bias (+ elu / log_softmax)
                    rec = sb.tile([P, nw, NH], F32, tag="rec")
                    nc.vector.reciprocal(out=rec[:],
                                         in_=accb[:, :, DH:DH + NH])
                    h = sb.tile([P, nw, DH], F32, tag="h")
                    nc.vector.tensor_tensor(
                        out=h[:].rearrange("p w (h c) -> p w h c", h=NH),
                        in0=accb[:, :, 0:DH].rearrange(
                            "p w (h c) -> p w h c", h=NH),
                        in1=rec[:, :, :, None].to_broadcast([P, nw, NH, CH]),
                        op=mybir.AluOpType.mult)
                    nc.vector.tensor_tensor(
                        out=h[:], in0=h[:],
                        in1=(b1s if layer == 1 else b2s)[:, None, :]
                        .to_broadcast([P, nw, DH]),
                        op=mybir.AluOpType.add)
                    if layer == 1:
                        # elu -> h ; then hp2 table rows per window
                        t1 = sb.tile([P, nw, DH], F32, tag="elu1")
                        nc.vector.tensor_scalar(
                            out=t1[:], in0=h[:], scalar1=0.0, scalar2=-1.0,
                            op0=mybir.AluOpType.max,
                            op1=mybir.AluOpType.add)
                        t2 = sb.tile([P, nw, DH], F32, tag="elu2")
                        nc.vector.tensor_scalar_min(out=t2[:], in0=h[:],
                                                    scalar1=0.0)
                        nc.scalar.activation(
                            out=t2[:].rearrange("p w c -> p (w c)"),
                            in_=t2[:].rearrange("p w c -> p (w c)"),
                            func=mybir.ActivationFunctionType.Exp)
                        nc.vector.tensor_tensor(out=h[:], in0=t1[:],
                                                in1=t2[:],
                                                op=mybir.AluOpType.add)
                        for w in range(w0, w1):
                            wl = w - w0
                            rows = P if w < nwin - 1 else last_rows
                            hTp = ps1.tile([D1, P], F32, tag="hTp")
                            nc.tensor.transpose(out=hTp[:], in_=h[:, wl, :],
                                                identity=ident[:])
                            hTb = sb.tile([D1, P], BF16, tag="hTb")
                            nc.vector.tensor_copy(out=hTb[:], in_=hTp[:])
                            p2 = ps1.tile([P, T2W], F32, tag="p2")
                            nc.tensor.matmul(p2[:], lhsT=hTb[:], rhs=w2s[:],
                                             start=True, stop=True)
                            t2row8 = sb.tile([P, D2], FP8, tag="t2row8")
                            nc.vector.tensor_copy(out=t2row8[:],
                                                  in_=p2[:, 0:D2])
                            a2row = sb.tile([P, 1], BF16, tag="a2row")
                            nc.scalar.copy(out=a2row[:],
                                           in_=p2[:, D2 + 1:D2 + 2])
                            nc.sync.dma_start(
                                out=t2loc[w * P:w * P + rows, 0:D2],
                                in_=t2row8[:rows, :])
                            nc.sync.dma_start(
                                out=ald2[w * P:w * P + rows, 0:1],
                                in_=a2row[:rows, :])
                    else:
                        # span-batched log_softmax rows -> out
                        mx = sb.tile([P, nw, 1], F32, tag="mx")
                        nc.vector.tensor_reduce(
                            out=mx[:], in_=h[:], axis=mybir.AxisListType.X,
                            op=mybir.AluOpType.max)
                        tt = sb.tile([P, nw, D2], F32, tag="tt")
                        nc.vector.tensor_tensor(
                            out=tt[:], in0=h[:],
                            in1=mx[:].to_broadcast([P, nw, D2]),
                            op=mybir.AluOpType.subtract)
                        ex = sb.tile([P, nw, D2], F32, tag="ex")
                        nc.scalar.activation(
                            out=ex[:].rearrange("p w c -> p (w c)"),
                            in_=tt[:].rearrange("p w c -> p (w c)"),
                            func=mybir.ActivationFunctionType.Exp)
                        s = sb.tile([P, nw, 1], F32, tag="s")
                        nc.vector.tensor_reduce(
                            out=s[:], in_=ex[:], axis=mybir.AxisListType.X,
                            op=mybir.AluOpType.add)
                        ls = sb.tile([P, nw, 1], F32, tag="ls")
                        nc.scalar.activation(
                            out=ls[:].rearrange("p w c -> p (w c)"),
                            in_=s[:].rearrange("p w c -> p (w c)"),
                            func=mybir.ActivationFunctionType.Ln)
                        res = sb.tile([P, nw, D2], F32, tag="res")
                        nc.vector.tensor_tensor(
                            out=res[:], in0=tt[:],
                            in1=ls[:].to_broadcast([P, nw, D2]),
                            op=mybir.AluOpType.subtract)
                        if w1 < nwin:
                            nc.sync.dma_start(
                                out=out[w0 * P:w1 * P, :].rearrange(
                                    "(w p) c -> p w c", p=P),
                                in_=res[:])
                        else:
                            for w in range(w0, w1):
                                wl = w - w0
                                rows = P if w < nwin - 1 else last_rows
                                nc.sync.dma_start(
                                    out=out[w * P:w * P + rows, :],
                                    in_=res[:rows, wl, :])

            edge_phase(1)
            nc.gpsimd.collective_compute(
                "AllGather", mybir.AluOpType.bypass,
                replica_groups=[list(range(ncores))],
                ins=[t2loc[:].opt()], outs=[t2full[:].opt()])
            edge_phase(2)

    nc.compile()
    return nc


# ------------------------------------------------------------------ runner
class SpmdRunner:
    def __init__(self, nc, n_cores):
        import jax
        from jax.sharding import Mesh, PartitionSpec
        from jax.experimental.shard_map import shard_map
        from concourse.bass2jax import (_bass_exec_p, partition_id_tensor,
                                        install_neuronx_cc_hook)
        install_neuronx_cc_hook()
        self.jax = jax
        self.n_cores = n_cores
        pname = nc.partition_id_tensor.name if nc.partition_id_tensor else None
        in_names, out_names, out_avals, zero_outs = [], [], [], []
        for alloc in nc.m.functions[0].allocations:
            if not isinstance(alloc, mybir.MemoryLocationSet):
                continue
            name = alloc.memorylocations[0].name
            if alloc.kind == "ExternalInput":
                if name != pname:
                    in_names.append(name)
            elif alloc.kind == "ExternalOutput":
                out_names.append(name)
                out_avals.append(jax.core.ShapedArray(
                    tuple(alloc.tensor_shape), mybir.dt.np(alloc.dtype)))
                zero_outs.append(np.zeros(tuple(alloc.tensor_shape),
                                          mybir.dt.np(alloc.dtype)))
        self.in_names, self.out_names = in_names, out_names
        self.out_avals, self.zero_outs = out_avals, zero_outs
        self.n_params = len(in_names)
        all_in = in_names + out_names + ([pname] if pname else [])

        def _body(*args):
            operands = list(args)
            if pname is not None:
                operands.append(partition_id_tensor())
            return tuple(_bass_exec_p.bind(
                *operands, out_avals=tuple(out_avals), in_names=tuple(all_in),
                out_names=tuple(out_names), lowering_input_output_aliases=(),
                sim_require_finite=True, sim_require_nnan=True, nc=nc))

        donate = tuple(range(self.n_params, self.n_params + len(out_avals)))
        devices = jax.devices()[:n_cores]
        self.mesh = Mesh(np.asarray(devices), ("core",))
        self.pspec = PartitionSpec("core")
        in_specs = (self.pspec,) * (self.n_params + len(out_avals))
        out_specs = (self.pspec,) * len(out_avals)
        self.sharded = jax.jit(
            shard_map(_body, mesh=self.mesh, in_specs=in_specs,
                      out_specs=out_specs, check_rep=False),
            donate_argnums=donate, keep_unused=True)

    def run(self, in_maps, reps=1):
        import time
        from jax.sharding import NamedSharding
        jax = self.jax
        sh = NamedSharding(self.mesh, self.pspec)
        per_core = [[np.asarray(m[name]) for name in self.in_names]
                    for m in in_maps]
        concat = [np.concatenate([per_core[c][i] for c in range(self.n_cores)],
                                 axis=0) for i in range(self.n_params)]
        dev_in = [jax.device_put(a, sh) for a in concat]
        best = float("inf")
        out_arrs = None
        for _ in range(reps):
            dz = [jax.device_put(
                np.zeros((self.n_cores * z.shape[0], *z.shape[1:]), z.dtype), sh)
                for z in self.zero_outs]
            for a in dz:
                a.block_until_ready()
            t0 = time.perf_counter_ns()
            out_arrs = self.sharded(*dev_in, *dz)
            for a in out_arrs:
                a.block_until_ready()
            best = min(best, time.perf_counter_ns() - t0)
        results = [
            {name: np.asarray(out_arrs[i]).reshape(
                self.n_cores, *self.out_avals[i].shape)[c]
             for i, name in enumerate(self.out_names)}
            for c in range(self.n_cores)]
        return results, best


# ----------------------------------------------------------------- kernel()
def make_cfg(N, E, F_IN, H1, C1, C2, ncores):
    nloc = N // ncores
    return dict(N=N, E=E, F_IN=F_IN, H1=H1, C1=C1, C2=C2, ncores=ncores,
                nloc=nloc, nwin=math.ceil(nloc / P), nrange=4)


DEFAULT_CFG = make_cfg(N=100000, E=1600000, F_IN=512, H1=8, C1=8, C2=16,
                       ncores=8)


def fold_weights(W1, a1_src, a1_dst, W2, a2_src, a2_dst, cfg):
    H1, C1 = cfg["H1"], cfg["C1"]
    W1r = W1.reshape(cfg["F_IN"], H1, C1)
    w1s = np.einsum("khc,hc->kh", W1r, a1_src)
    w1d = np.einsum("khc,hc->kh", W1r, a1_dst)
    W1e = np.concatenate([W1, w1s, w1d], axis=1).astype(bf16)
    w2s = W2 @ a2_src[0]
    w2d = W2 @ a2_dst[0]
    W2e = np.concatenate([W2, w2s[:, None], w2d[:, None]], axis=1).astype(bf16)
    return W1e, W2e


_CACHE = {}


def prepare(inputs, cfg=DEFAULT_CFG, reps=1):
    x = np.asarray(inputs["x"], np.float32)
    edge_index = np.asarray(inputs["edge_index"])
    W1 = np.asarray(inputs["W1"], np.float32)
    W2 = np.asarray(inputs["W2"], np.float32)
    b1 = np.asarray(inputs["b1"], np.float32)
    b2 = np.asarray(inputs["b2"], np.float32)
    a1s = np.asarray(inputs["a1_src"], np.float32)
    a1d = np.asarray(inputs["a1_dst"], np.float32)
    a2s = np.asarray(inputs["a2_src"], np.float32)
    a2d = np.asarray(inputs["a2_dst"], np.float32)

    per_core_idx, grid = preprocess(edge_index, cfg)
    key = (cfg["N"], grid.w_tot, tuple(int(v) for v in grid.r_tot))
    if key not in _CACHE:
        nc = build_nc(cfg, grid)
        _CACHE[key] = (nc, SpmdRunner(nc, cfg["ncores"]))
    nc, runner = _CACHE[key]

    W1e, W2e = fold_weights(W1, a1s, a1d, W2, a2s, a2d, cfg)
    b1rep = np.tile(b1[None, :], (P, 1)).astype(np.float32)
    b2rep = np.tile(b2[None, :], (P, 1)).astype(np.float32)
    a1srep = np.tile(a1s.reshape(1, -1), (P, 1)).astype(np.float32)
    a2srep = np.tile(a2s.reshape(1, -1), (P, 1)).astype(np.float32)
    nloc = cfg["nloc"]
    in_maps = []
    for c in range(cfg["ncores"]):
        m = dict(per_core_idx[c])
        m["xT"] = np.ascontiguousarray(
            x[c * nloc:(c + 1) * nloc, :].T).astype(bf16)
        m["W1e"], m["W2e"] = W1e, W2e
        m["b1r"], m["b2r"] = b1rep, b2rep
        m["a1sr"], m["a2sr"] = a1srep, a2srep
        in_maps.append(m)
    return runner, in_maps


def kernel_timed(inputs, reps=1):
    cfg = DEFAULT_CFG
    runner, in_maps = prepare(inputs, cfg, reps)
    results, best_ns = runner.run(in_maps, reps=reps)
    out = np.concatenate([results[c]["out"] for c in range(cfg["ncores"])],
                         axis=0)
    return out, best_ns


def kernel(**inputs):
    out, _ = kernel_timed(inputs, reps=1)
    return out


# revision 45
# speedup vs baseline: 1.6950x; 1.6950x over previous
"""2-layer GAT (gnn_message_passing) on 8 Trainium2 NeuronCores.

Strategy (per sharding hint): nodes are partitioned contiguously across the 8
cores (12500 each). Edges (incl. self-loops) are sharded by destination core,
sorted by destination window (128 dst nodes) and source range (quarter of the
node space, so gather indices fit int16), and padded to a per-(window,range)
variable tile grid (tiles = max over cores of ceil(count/128), so all cores
share one SPMD program).

Per layer: a dense phase computes per-node transformed features hp = x @ W
(and the al_dst attention-logit half, folded into the weight matrix), writes
hp as fp8 into a 256B-strided row table and al_dst into a bf16 side table,
and an AllGather replicates the hp table to every core. The edge phase
gathers fp8 hp rows by edge source and bf16 al_dst by edge destination
(custom Ant dma_gather; the per-queue SDMA drain rate for small random reads
is the kernel bottleneck, so payloads are minimized — al_src is recomputed
on-device from the gathered fp8 rows instead of being gathered — and the
gather stream is byte-balanced across all 4 SWDGE queues). It then forms
ee = exp(leaky_relu(al_s + al_d)) per edge, multiplies messages, and
scatter-adds per destination window with a one-hot selection matmul on the PE
(which also accumulates the softmax denominators). Normalization / ELU /
log_softmax tails are batched per span of windows; output rows are contiguous
per window, so no scatter is needed on the way out.
"""
import math
import numpy as np
import ml_dtypes

import concourse.bacc as bacc
import concourse.mybir as mybir
import concourse.tile as tile
from concourse import ap_utils

bf16 = ml_dtypes.bfloat16
F32 = mybir.dt.float32
BF16 = mybir.dt.bfloat16
FP8 = mybir.dt.float8e4
I16 = mybir.dt.int16
I32 = mybir.dt.int32

P = 128
MAX_IDX_PER_GATHER = 3840   # DMA desc ring: <=~4080 idxs per gather inst
SLOPE = 0.2
NQ = 4                      # SWDGE queues for gathers


# ---------------------------------------------------------------- dma_gather
def dma_gather_raw(eng, out_ap, in_ap, idxs_ap, num_idxs, elem_size,
                   elem_step=None, queue_num=0):
    """BassGpSimd.dma_gather (DRAM src, non-transpose) minus the
    elem_size%256B assert (transpose-only restriction, see q7 source) and
    with single_packet=False (large single packets wedge the SDMA)."""
    assert idxs_ap.dtype == mybir.dt.int16
    assert in_ap.dtype == out_ap.dtype
    elem_size_bytes = elem_size * mybir.dt.size(in_ap.dtype)
    assert elem_size_bytes > 0
    if elem_step is None:
        elem_step = elem_size
    assert ap_utils.ap_is_contiguous(in_ap.ap[1:])
    assert ap_utils.ap_is_contiguous(out_ap.ap[1:])
    assert ap_utils.ap_is_contiguous(idxs_ap.ap[1:])
    assert in_ap.ap[0][0] == elem_step
    assert in_ap.ap[-1][1] == elem_size
    assert out_ap.ap[-1][1] == elem_size
    assert num_idxs <= MAX_IDX_PER_GATHER + 256
    stride_bytes = elem_step * mybir.dt.size(in_ap.dtype)
    assert stride_bytes % 256 == 0 and stride_bytes // 256 < 256
    _in_ap = eng.lower_ap_dma(in_ap, for_custom_bir_dma=True)
    _idxs_ap = eng.lower_ap(idxs_ap)
    _out_ap = eng.lower_ap(out_ap)
    return eng.add_instruction(
        mybir.InstDMAGatherAnt(
            name=eng.bass.get_next_instruction_name(),
            ins=[*_in_ap, _idxs_ap, eng.lower_val_access(eng.to_reg(num_idxs))],
            outs=[_out_ap],
            transpose=False,
            num_idxs=num_idxs,
            elem_size=elem_size,
            stride_bytes_256=stride_bytes // 256,
            gen_mode=0,
            single_packet=False,
            queue_num=queue_num,
            sbuf_tokens_per_rank=0,
            sbuf_free_dim_per_rank=0,
            sbuf_free_dim_pad_per_rank=0,
            sbuf_byte_offset=0,
        )
    )


# ------------------------------------------------------------- host preproc
def _wrap_idx(seq16):
    """[L] int -> [128, L//16] int16 in dma_gather idx layout
    (idx j at lane j%16 col j//16, replicated to 8 lane groups)."""
    L = seq16.shape[0]
    w = seq16.reshape(L // 16, 16).T                           # [16,K]
    w = np.tile(w, (8, 1))                                     # [128,K]
    return np.ascontiguousarray(w.astype(np.int16))


class Grid:
    """Static (shared across cores) variable-tile grid for the edge phase."""

    def __init__(self, counts, nwin, nrange):
        # counts: [ncores, nwin, nrange]
        self.nwin, self.nrange = nwin, nrange
        cmax = counts.max(axis=0)                              # [nwin,nrange]
        self.tiles = np.maximum(1, np.ceil(cmax / P).astype(np.int64))
        self.wtiles = self.tiles.sum(axis=1)                   # [nwin]
        # greedy window spans: per-range idx count within a span <= cap
        cap_tiles = MAX_IDX_PER_GATHER // P
        spans = []
        w0 = 0
        while w0 < nwin:
            w1 = w0 + 1
            while w1 < nwin:
                nxt = self.tiles[w0:w1 + 1].sum(axis=0).max()
                if nxt > cap_tiles:
                    break
                w1 += 1
            spans.append((w0, w1))
            w0 = w1
        self.spans = spans
        # per-range per-window tile offset within the whole layout
        self.r_off = np.zeros((nwin, nrange), np.int64)        # in tiles
        for r in range(nrange):
            self.r_off[1:, r] = np.cumsum(self.tiles[:-1, r])
        self.r_tot = self.tiles.sum(axis=0)                    # [nrange]
        # dstrel per-window offsets (in tiles, over concat of all ranges)
        self.w_off = np.zeros(nwin, np.int64)
        self.w_off[1:] = np.cumsum(self.wtiles[:-1])
        self.w_tot = int(self.wtiles.sum())
        self.max_wtiles = int(self.wtiles.max())


def preprocess(edge_index, cfg):
    """Sort/pad edges into the variable (core, window, range, tile) grid."""
    N, ncores, nloc, nwin, nrange = (cfg["N"], cfg["ncores"], cfg["nloc"],
                                     cfg["nwin"], cfg["nrange"])
    rng_sz = N // nrange
    loops = np.arange(N, dtype=np.int64)
    src = np.concatenate([edge_index[0].astype(np.int64), loops])
    dst = np.concatenate([edge_index[1].astype(np.int64), loops])
    core = dst // nloc
    dst_loc = dst - core * nloc
    w = dst_loc // P
    dst_rel = dst_loc - w * P
    r = src // rng_sz
    src_rel = src - r * rng_sz
    key = (core * nwin + w) * nrange + r
    counts = np.bincount(key, minlength=ncores * nwin * nrange).reshape(
        ncores, nwin, nrange)
    grid = Grid(counts, nwin, nrange)

    order = np.argsort(key, kind="stable")
    ks = key[order]
    starts = np.zeros(ncores * nwin * nrange + 1, np.int64)
    np.cumsum(counts.reshape(-1), out=starts[1:])
    pos = np.arange(len(ks)) - starts[ks]
    # window-local tile offset of range r: sum of tiles[w, :r]
    tloc = np.zeros((nwin, nrange), np.int64)
    tloc[:, 1:] = np.cumsum(grid.tiles[:, :-1], axis=1)
    # per (r) gather layout: [r_tot[r]*P] slots; edges of (c,w,r) at
    # r_off[w,r]*P + pos
    per_core = []
    srcrel_o = src_rel[order]
    dstrel_o = dst_rel[order]
    core_o = core[order]
    w_o = w[order]
    r_o = r[order]
    for c in range(ncores):
        m = {}
        sel = core_o == c
        sw, sr, sp = w_o[sel], r_o[sel], pos[sel]
        ssrc, sdst = srcrel_o[sel], dstrel_o[sel]
        for rr in range(nrange):
            rs = sr == rr
            tot = int(grid.r_tot[rr]) * P
            slot = grid.r_off[sw[rs], rr] * P + sp[rs]
            buf = np.zeros(tot, np.int16)
            buf[slot] = ssrc[rs].astype(np.int16)
            m[f"iA{rr}"] = _wrap_idx(buf)
            bufd = np.zeros(tot, np.int16)
            bufd[slot] = (sw[rs] * P + sdst[rs]).astype(np.int16)
            m[f"iB{rr}"] = _wrap_idx(bufd)
        # dstrel layout [128, w_tot] (per window: ranges concat in r order)
        dr = np.full(grid.w_tot * P, -1.0, np.float32)
        gslot = (grid.w_off[sw] + tloc[sw, sr]) * P + sp
        dr[gslot] = sdst.astype(np.float32)
        # device layout [128, w_tot]: slot (tile t, lane p) -> [p, t]
        m["dstrel"] = np.ascontiguousarray(
            dr.reshape(grid.w_tot, P).T.astype(bf16))
        per_core.append(m)
    return per_core, grid


# ------------------------------------------------------------- device build
def build_nc(cfg, grid):
    N, ncores, nloc, nwin, nrange = (cfg["N"], cfg["ncores"], cfg["nloc"],
                                     cfg["nwin"], cfg["nrange"])
    F_IN, H1, C1, C2 = cfg["F_IN"], cfg["H1"], cfg["C1"], cfg["C2"]
    D1 = H1 * C1                   # 64
    A1 = D1 + 2 * H1               # 80: [hp | al_s | al_d]
    T1W = D1 + H1                  # 72 table row (hp | al_s)
    D2 = C2                        # 16
    T2W = D2 + 2                   # 18 table row (hp2 | al_s2 | pad)
    kchunks = F_IN // P
    last_rows = nloc - (nwin - 1) * P
    tiles, spans = grid.tiles, grid.spans
    tloc = np.zeros((nwin, nrange), np.int64)
    tloc[:, 1:] = np.cumsum(tiles[:, :-1], axis=1)

    nc = bacc.Bacc("TRN2", target_bir_lowering=False, num_devices=ncores,
                   num_swdge_queues=NQ)
    xT = nc.dram_tensor("xT", [F_IN, nloc], BF16, kind="ExternalInput")
    W1e = nc.dram_tensor("W1e", [F_IN, A1], BF16, kind="ExternalInput")
    W2e = nc.dram_tensor("W2e", [D1, T2W], BF16, kind="ExternalInput")
    b1r = nc.dram_tensor("b1r", [P, D1], F32, kind="ExternalInput")
    b2r = nc.dram_tensor("b2r", [P, D2], F32, kind="ExternalInput")
    a1sr = nc.dram_tensor("a1sr", [P, D1], F32, kind="ExternalInput")
    a2sr = nc.dram_tensor("a2sr", [P, D2], F32, kind="ExternalInput")
    iA = [nc.dram_tensor(f"iA{rr}", [P, int(grid.r_tot[rr]) * 8], I16,
                         kind="ExternalInput") for rr in range(nrange)]
    iB = [nc.dram_tensor(f"iB{rr}", [P, int(grid.r_tot[rr]) * 8], I16,
                         kind="ExternalInput") for rr in range(nrange)]
    dstrel = nc.dram_tensor("dstrel", [P, grid.w_tot], BF16,
                            kind="ExternalInput")
    out = nc.dram_tensor("out", [nloc, D2], F32, kind="ExternalOutput")

    qbytes = [0] * NQ

    def next_q(nbytes):
        q = min(range(NQ), key=lambda i: qbytes[i])
        qbytes[q] += nbytes
        return q

    with tile.TileContext(nc) as tc:
        with (
            tc.tile_pool(name="const", bufs=1) as cpool,
            tc.tile_pool(name="sbuf", bufs=2) as sb,
            tc.tile_pool(name="selp", bufs=3) as slp,
            tc.tile_pool(name="gat", bufs=3) as gp,
            tc.tile_pool(name="psum", bufs=2, space="PSUM") as ps,
            tc.tile_pool(name="psum1", bufs=2, space="PSUM") as ps1,
            tc.tile_pool(name="dram", bufs=1, space="DRAM") as dr,
        ):
            # layer-1 table row (256B stride): [64 fp8 hp | 8 bf16 al_s | pad]
            t1loc = dr.tile([nloc, 2 * P], FP8)
            t1full = dr.tile([N, 2 * P], FP8, addr_space="Shared")
            ald1 = dr.tile([nloc, 2 * P], FP8)
            # layer-2 table row (256B stride): [16 fp8 hp2 | 1 bf16 al_s2]
            t2loc = dr.tile([nloc, 2 * P], FP8)
            t2full = dr.tile([N, 2 * P], FP8, addr_space="Shared")
            ald2 = dr.tile([nloc, P], BF16)

            # ---- static constants
            w1s = cpool.tile([P, kchunks, A1], BF16)
            nc.sync.dma_start(out=w1s[:], in_=W1e[:].rearrange(
                "(c p) a -> p c a", p=P))
            w2s = cpool.tile([D1, T2W], BF16)
            nc.sync.dma_start(out=w2s[:], in_=W2e[:])
            b1s = cpool.tile([P, D1], F32)
            nc.sync.dma_start(out=b1s[:], in_=b1r[:])
            b2s = cpool.tile([P, D2], F32)
            nc.sync.dma_start(out=b2s[:], in_=b2r[:])
            a1ss = cpool.tile([P, D1], F32)
            nc.sync.dma_start(out=a1ss[:], in_=a1sr[:])
            a2ss = cpool.tile([P, D2], F32)
            nc.sync.dma_start(out=a2ss[:], in_=a2sr[:])
            iota_f = cpool.tile([P, grid.max_wtiles, P], BF16)
            nc.gpsimd.iota(iota_f[:], pattern=[[0, grid.max_wtiles], [1, P]],
                           base=0, channel_multiplier=0,
                           allow_small_or_imprecise_dtypes=True)
            ident = cpool.tile([P, P], F32)
            from concourse.masks import make_identity
            make_identity(nc, ident[:])

            # preload all gather idx tables + dstrel (shared by both layers)
            iAs, iBs = [], []
            for rr in range(nrange):
                kr = int(grid.r_tot[rr]) * 8
                ta = cpool.tile([P, kr], I16, name=f"iAs{rr}")
                nc.sync.dma_start(out=ta[:], in_=iA[rr][:])
                iAs.append(ta)
                tb = cpool.tile([P, kr], I16, name=f"iBs{rr}")
                nc.sync.dma_start(out=tb[:], in_=iB[rr][:])
                iBs.append(tb)
            dres = cpool.tile([P, grid.w_tot], BF16)
            nc.sync.dma_start(out=dres[:], in_=dstrel[:])

            # ---- phase A: tables for layer 1
            for b in range(nwin):
                r0 = b * P
                rows = P if b < nwin - 1 else last_rows
                xa = sb.tile([P, kchunks, P], BF16, tag="xa")
                nc.sync.dma_start(
                    out=xa[:, :, :rows],
                    in_=xT[:, r0:r0 + rows].rearrange("(c p) r -> p c r", p=P))
                pA = ps.tile([P, A1], F32, tag="pA")
                for c in range(kchunks):
                    nc.tensor.matmul(pA[:], lhsT=xa[:, c, :], rhs=w1s[:, c, :],
                                     start=(c == 0), stop=(c == kchunks - 1))
                t1row8 = sb.tile([P, D1], FP8, tag="t1row8")
                nc.vector.tensor_copy(out=t1row8[:], in_=pA[:, 0:D1])
                a1row = sb.tile([P, H1], FP8, tag="a1row")
                nc.scalar.copy(out=a1row[:], in_=pA[:, T1W:A1])
                nc.sync.dma_start(out=t1loc[r0:r0 + rows, 0:D1],
                                  in_=t1row8[:rows, :])
                nc.sync.dma_start(out=ald1[r0:r0 + rows, 0:H1],
                                  in_=a1row[:rows, :])

            # ---- allgather T1
            nc.gpsimd.collective_compute(
                "AllGather", mybir.AluOpType.bypass,
                replica_groups=[list(range(ncores))],
                ins=[t1loc[:].opt()], outs=[t1full[:].opt()])

            # ---- edge phases
            def edge_phase(layer):
                tfull = t1full if layer == 1 else t2full
                ald = ald1 if layer == 1 else ald2
                DH = D1 if layer == 1 else D2       # message width
                NH = H1 if layer == 1 else 1        # heads
                CH = DH // NH
                asrc = a1ss if layer == 1 else a2ss
                # fp8 table rows: [DH fp8 hp]; al_s recomputed on-device
                TW = DH
                ebytes = TW
                rng_rows = N // nrange
                for (w0, w1) in spans:
                    span_t = tiles[w0:w1].sum(axis=0)        # per-range tiles
                    rhs_g = []
                    for rr in range(nrange):
                        st = int(span_t[rr])
                        nidx = st * P
                        t_off = int(grid.r_off[w0, rr])      # tiles
                        hg = gp.tile([P, st, TW], FP8, tag=f"hg{rr}")
                        dma_gather_raw(
                            nc.gpsimd, hg[:],
                            tfull[rr * rng_rows:(rr + 1) * rng_rows, 0:TW],
                            iAs[rr][:, t_off * 8:(t_off + st) * 8],
                            nidx, TW, elem_step=2 * P,
                            queue_num=next_q(nidx * ebytes))
                        adt = FP8 if layer == 1 else BF16
                        ag = gp.tile([P, st, NH], adt, tag=f"ag{rr}")
                        dma_gather_raw(
                            nc.gpsimd, ag[:],
                            ald[:, 0:NH],
                            iBs[rr][:, t_off * 8:(t_off + st) * 8],
                            nidx, NH,
                            elem_step=2 * P if layer == 1 else P,
                            queue_num=next_q(nidx * NH * mybir.dt.size(adt)))
                        # al_s = sum_c hp[h,c]*a_src[h,c] from the fp8 rows
                        # (msg tile doubles as scratch for the products)
                        msg = gp.tile([P, st, DH + NH], BF16, tag=f"msg{rr}")
                        nc.vector.tensor_tensor(
                            out=msg[:, :, 0:DH].rearrange(
                                "p t (h c) -> p t h c", h=NH),
                            in0=hg[:, :, 0:DH].rearrange(
                                "p t (h c) -> p t h c", h=NH),
                            in1=asrc[:, None, :].rearrange(
                                "p t (h c) -> p t h c", h=NH).to_broadcast(
                                [P, st, NH, CH]),
                            op=mybir.AluOpType.mult)
                        z = gp.tile([P, st, NH], BF16, tag=f"zz{rr}")
                        with nc.allow_low_precision(
                                reason="8-wide al_s dot; bf16 ee is enough"):
                            nc.vector.tensor_reduce(
                                out=z[:, :, :, None],
                                in_=msg[:, :, 0:DH].rearrange(
                                    "p t (h c) -> p t h c", h=NH),
                                axis=mybir.AxisListType.X,
                                op=mybir.AluOpType.add)
                        # ee = exp(lrelu(al_s + al_d))
                        nc.vector.tensor_tensor(
                            out=z[:], in0=z[:], in1=ag[:],
                            op=mybir.AluOpType.add)
                        zf = z[:].rearrange("p t h -> p (t h)")
                        nc.scalar.activation(
                            out=zf, in_=zf,
                            func=mybir.ActivationFunctionType.Lrelu,
                            alpha=SLOPE)
                        nc.scalar.activation(
                            out=zf, in_=zf,
                            func=mybir.ActivationFunctionType.Exp)
                        # messages into msg: [hp*ee | ee]
                        nc.vector.tensor_tensor(
                            out=msg[:, :, 0:DH].rearrange(
                                "p t (h c) -> p t h c", h=NH),
                            in0=hg[:, :, 0:DH].rearrange(
                                "p t (h c) -> p t h c", h=NH),
                            in1=z[:, :, :, None].to_broadcast(
                                [P, st, NH, CH]),
                            op=mybir.AluOpType.mult)
                        nc.vector.tensor_copy(
                            out=msg[:, :, DH:DH + NH], in_=z[:])
                        rhs_g.append(msg)
                    d_off = int(grid.w_off[w0])
                    dre = dres

                    nw = w1 - w0
                    accb = sb.tile([P, nw, DH + NH], F32, tag="accb")
                    for w in range(w0, w1):
                        wl = w - w0
                        wt = int(grid.wtiles[w])
                        sel = slp.tile([P, wt, P], BF16, tag="sel")
                        dre_o = int(grid.w_off[w])
                        nc.vector.tensor_tensor(
                            out=sel[:],
                            in0=iota_f[:, :wt, :],
                            in1=dre[:, dre_o:dre_o + wt, None].to_broadcast(
                                [P, wt, P]),
                            op=mybir.AluOpType.is_equal)
                        acc = ps.tile([P, DH + NH], F32, tag="acc")
                        nmm = wt
                        i = 0
                        for rr in range(nrange):
                            st_w = int(tiles[w, rr])
                            gt0 = int(grid.r_off[w, rr] - grid.r_off[w0, rr])
                            sl0 = int(tloc[w, rr])
                            for t in range(st_w):
                                nc.tensor.matmul(
                                    acc[:],
                                    lhsT=sel[:, sl0 + t, :],
                                    rhs=rhs_g[rr][:, gt0 + t, 0:DH + NH],
                                    start=(i == 0), stop=(i == nmm - 1))
                                i += 1
                        nc.scalar.copy(out=accb[:, wl, :], in_=acc[:])

                    # span-batched normalization + bias (+ elu / log_softmax)
                    rec = sb.tile([P, nw, NH], F32, tag="rec")
                    nc.vector.reciprocal(out=rec[:],
                                         in_=accb[:, :, DH:DH + NH])
                    h = sb.tile([P, nw, DH], F32, tag="h")
                    nc.vector.tensor_tensor(
                        out=h[:].rearrange("p w (h c) -> p w h c", h=NH),
                        in0=accb[:, :, 0:DH].rearrange(
                            "p w (h c) -> p w h c", h=NH),
                        in1=rec[:, :, :, None].to_broadcast([P, nw, NH, CH]),
                        op=mybir.AluOpType.mult)
                    nc.vector.tensor_tensor(
                        out=h[:], in0=h[:],
                        in1=(b1s if layer == 1 else b2s)[:, None, :]
                        .to_broadcast([P, nw, DH]),
                        op=mybir.AluOpType.add)
                    if layer == 1:
                        # elu -> h ; then hp2 table rows per window
                        t1 = sb.tile([P, nw, DH], F32, tag="elu1")
                        nc.vector.tensor_scalar(
                            out=t1[:], in0=h[:], scalar1=0.0, scalar2=-1.0,
                            op0=mybir.AluOpType.max,
                            op1=mybir.AluOpType.add)
                        t2 = sb.tile([P, nw, DH], F32, tag="elu2")
                        nc.vector.tensor_scalar_min(out=t2[:], in0=h[:],
                                                    scalar1=0.0)
                        nc.scalar.activation(
                            out=t2[:].rearrange("p w c -> p (w c)"),
                            in_=t2[:].rearrange("p w c -> p (w c)"),
                            func=mybir.ActivationFunctionType.Exp)
                        nc.vector.tensor_tensor(out=h[:], in0=t1[:],
                                                in1=t2[:],
                                                op=mybir.AluOpType.add)
                        for w in range(w0, w1):
                            wl = w - w0
                            rows = P if w < nwin - 1 else last_rows
                            hTp = ps1.tile([D1, P], F32, tag="hTp")
                            nc.tensor.transpose(out=hTp[:], in_=h[:, wl, :],
                                                identity=ident[:])
                            hTb = sb.tile([D1, P], BF16, tag="hTb")
                            nc.vector.tensor_copy(out=hTb[:], in_=hTp[:])
                            p2 = ps1.tile([P, T2W], F32, tag="p2")
                            nc.tensor.matmul(p2[:], lhsT=hTb[:], rhs=w2s[:],
                                             start=True, stop=True)
                            t2row8 = sb.tile([P, D2], FP8, tag="t2row8")
                            nc.vector.tensor_copy(out=t2row8[:],
                                                  in_=p2[:, 0:D2])
                            a2row = sb.tile([P, 1], BF16, tag="a2row")
                            nc.scalar.copy(out=a2row[:],
                                           in_=p2[:, D2 + 1:D2 + 2])
                            nc.sync.dma_start(
                                out=t2loc[w * P:w * P + rows, 0:D2],
                                in_=t2row8[:rows, :])
                            nc.sync.dma_start(
                                out=ald2[w * P:w * P + rows, 0:1],
                                in_=a2row[:rows, :])
                    else:
                        # span-batched log_softmax rows -> out
                        mx = sb.tile([P, nw, 1], F32, tag="mx")
                        nc.vector.tensor_reduce(
                            out=mx[:], in_=h[:], axis=mybir.AxisListType.X,
                            op=mybir.AluOpType.max)
                        tt = sb.tile([P, nw, D2], F32, tag="tt")
                        nc.vector.tensor_tensor(
                            out=tt[:], in0=h[:],
                            in1=mx[:].to_broadcast([P, nw, D2]),
                            op=mybir.AluOpType.subtract)
                        ex = sb.tile([P, nw, D2], F32, tag="ex")
                        nc.scalar.activation(
                            out=ex[:].rearrange("p w c -> p (w c)"),
                            in_=tt[:].rearrange("p w c -> p (w c)"),
                            func=mybir.ActivationFunctionType.Exp)
                        s = sb.tile([P, nw, 1], F32, tag="s")
                        nc.vector.tensor_reduce(
                            out=s[:], in_=ex[:], axis=mybir.AxisListType.X,
                            op=mybir.AluOpType.add)
                        ls = sb.tile([P, nw, 1], F32, tag="ls")
                        nc.scalar.activation(
                            out=ls[:].rearrange("p w c -> p (w c)"),
                            in_=s[:].rearrange("p w c -> p (w c)"),
                            func=mybir.ActivationFunctionType.Ln)
                        res = sb.tile([P, nw, D2], F32, tag="res")
                        nc.vector.tensor_tensor(
                            out=res[:], in0=tt[:],
                            in1=ls[:].to_broadcast([P, nw, D2]),
                            op=mybir.AluOpType.subtract)
                        if w1 < nwin:
                            nc.sync.dma_start(
                                out=out[w0 * P:w1 * P, :].rearrange(
                                    "(w p) c -> p w c", p=P),
                                in_=res[:])
                        else:
                            for w in range(w0, w1):
                                wl = w - w0
                                rows = P if w < nwin - 1 else last_rows
                                nc.sync.dma_start(
                                    out=out[w * P:w * P + rows, :],
                                    in_=res[:rows, wl, :])

            edge_phase(1)
            nc.gpsimd.collective_compute(
                "AllGather", mybir.AluOpType.bypass,
                replica_groups=[list(range(ncores))],
                ins=[t2loc[:].opt()], outs=[t2full[:].opt()])
            edge_phase(2)

    nc.compile()
    return nc


# ------------------------------------------------------------------ runner
class SpmdRunner:
    def __init__(self, nc, n_cores):
        import jax
        from jax.sharding import Mesh, PartitionSpec
        from jax.experimental.shard_map import shard_map
        from concourse.bass2jax import (_bass_exec_p, partition_id_tensor,
                                        install_neuronx_cc_hook)
        install_neuronx_cc_hook()
        self.jax = jax
        self.n_cores = n_cores
        pname = nc.partition_id_tensor.name if nc.partition_id_tensor else None
        in_names, out_names, out_avals, zero_outs = [], [], [], []
        for alloc in nc.m.functions[0].allocations:
            if not isinstance(alloc, mybir.MemoryLocationSet):
                continue
            name = alloc.memorylocations[0].name
            if alloc.kind == "ExternalInput":
                if name != pname:
                    in_names.append(name)
            elif alloc.kind == "ExternalOutput":
                out_names.append(name)
                out_avals.append(jax.core.ShapedArray(
                    tuple(alloc.tensor_shape), mybir.dt.np(alloc.dtype)))
                zero_outs.append(np.zeros(tuple(alloc.tensor_shape),
                                          mybir.dt.np(alloc.dtype)))
        self.in_names, self.out_names = in_names, out_names
        self.out_avals, self.zero_outs = out_avals, zero_outs
        self.n_params = len(in_names)
        all_in = in_names + out_names + ([pname] if pname else [])

        def _body(*args):
            operands = list(args)
            if pname is not None:
                operands.append(partition_id_tensor())
            return tuple(_bass_exec_p.bind(
                *operands, out_avals=tuple(out_avals), in_names=tuple(all_in),
                out_names=tuple(out_names), lowering_input_output_aliases=(),
                sim_require_finite=True, sim_require_nnan=True, nc=nc))

        donate = tuple(range(self.n_params, self.n_params + len(out_avals)))
        devices = jax.devices()[:n_cores]
        self.mesh = Mesh(np.asarray(devices), ("core",))
        self.pspec = PartitionSpec("core")
        in_specs = (self.pspec,) * (self.n_params + len(out_avals))
        out_specs = (self.pspec,) * len(out_avals)
        self.sharded = jax.jit(
            shard_map(_body, mesh=self.mesh, in_specs=in_specs,
                      out_specs=out_specs, check_rep=False),
            donate_argnums=donate, keep_unused=True)

    def run(self, in_maps, reps=1):
        import time
        from jax.sharding import NamedSharding
        jax = self.jax
        sh = NamedSharding(self.mesh, self.pspec)
        per_core = [[np.asarray(m[name]) for name in self.in_names]
                    for m in in_maps]
        concat = [np.concatenate([per_core[c][i] for c in range(self.n_cores)],
                                 axis=0) for i in range(self.n_params)]
        dev_in = [jax.device_put(a, sh) for a in concat]
        best = float("inf")
        out_arrs = None
        for _ in range(reps):
            dz = [jax.device_put(
                np.zeros((self.n_cores * z.shape[0], *z.shape[1:]), z.dtype), sh)
                for z in self.zero_outs]
            for a in dz:
                a.block_until_ready()
            t0 = time.perf_counter_ns()
            out_arrs = self.sharded(*dev_in, *dz)
            for a in out_arrs:
                a.block_until_ready()
            best = min(best, time.perf_counter_ns() - t0)
        results = [
            {name: np.asarray(out_arrs[i]).reshape(
                self.n_cores, *self.out_avals[i].shape)[c]
             for i, name in enumerate(self.out_names)}
            for c in range(self.n_cores)]
        return results, best


# ----------------------------------------------------------------- kernel()
def make_cfg(N, E, F_IN, H1, C1, C2, ncores):
    nloc = N // ncores
    return dict(N=N, E=E, F_IN=F_IN, H1=H1, C1=C1, C2=C2, ncores=ncores,
                nloc=nloc, nwin=math.ceil(nloc / P), nrange=4)


DEFAULT_CFG = make_cfg(N=100000, E=1600000, F_IN=512, H1=8, C1=8, C2=16,
                       ncores=8)


def fold_weights(W1, a1_src, a1_dst, W2, a2_src, a2_dst, cfg):
    H1, C1 = cfg["H1"], cfg["C1"]
    W1r = W1.reshape(cfg["F_IN"], H1, C1)
    w1s = np.einsum("khc,hc->kh", W1r, a1_src)
    w1d = np.einsum("khc,hc->kh", W1r, a1_dst)
    W1e = np.concatenate([W1, w1s, w1d], axis=1).astype(bf16)
    w2s = W2 @ a2_src[0]
    w2d = W2 @ a2_dst[0]
    W2e = np.concatenate([W2, w2s[:, None], w2d[:, None]], axis=1).astype(bf16)
    return W1e, W2e


_CACHE = {}


def prepare(inputs, cfg=DEFAULT_CFG, reps=1):
    x = np.asarray(inputs["x"], np.float32)
    edge_index = np.asarray(inputs["edge_index"])
    W1 = np.asarray(inputs["W1"], np.float32)
    W2 = np.asarray(inputs["W2"], np.float32)
    b1 = np.asarray(inputs["b1"], np.float32)
    b2 = np.asarray(inputs["b2"], np.float32)
    a1s = np.asarray(inputs["a1_src"], np.float32)
    a1d = np.asarray(inputs["a1_dst"], np.float32)
    a2s = np.asarray(inputs["a2_src"], np.float32)
    a2d = np.asarray(inputs["a2_dst"], np.float32)

    per_core_idx, grid = preprocess(edge_index, cfg)
    key = (cfg["N"], grid.w_tot, tuple(int(v) for v in grid.r_tot))
    if key not in _CACHE:
        nc = build_nc(cfg, grid)
        _CACHE[key] = (nc, SpmdRunner(nc, cfg["ncores"]))
    nc, runner = _CACHE[key]

    W1e, W2e = fold_weights(W1, a1s, a1d, W2, a2s, a2d, cfg)
    b1rep = np.tile(b1[None, :], (P, 1)).astype(np.float32)
    b2rep = np.tile(b2[None, :], (P, 1)).astype(np.float32)
    a1srep = np.tile(a1s.reshape(1, -1), (P, 1)).astype(np.float32)
    a2srep = np.tile(a2s.reshape(1, -1), (P, 1)).astype(np.float32)
    nloc = cfg["nloc"]
    in_maps = []
    for c in range(cfg["ncores"]):
        m = dict(per_core_idx[c])
        m["xT"] = np.ascontiguousarray(
            x[c * nloc:(c + 1) * nloc, :].T).astype(bf16)
        m["W1e"], m["W2e"] = W1e, W2e
        m["b1r"], m["b2r"] = b1rep, b2rep
        m["a1sr"], m["a2sr"] = a1srep, a2srep
        in_maps.append(m)
    return runner, in_maps


def kernel_timed(inputs, reps=1):
    cfg = DEFAULT_CFG
    runner, in_maps = prepare(inputs, cfg, reps)
    results, best_ns = runner.run(in_maps, reps=reps)
    out = np.concatenate([results[c]["out"] for c in range(cfg["ncores"])],
                         axis=0)
    return out, best_ns


def kernel(**inputs):
    out, _ = kernel_timed(inputs, reps=1)
    return out
